# revision 34
# baseline (speedup 1.0000x reference)
"""Trainium2 Bass kernel for the gnn_message_passing LoopModel.

Reference computation (per edge e, corners l/r from edge_corner):
    CF[n]    = mean over pairs (n, e') of x[e']          (segment mean)
    out[e]   = relu(W1 @ x[e] + W2 @ CF[l_e] + W3 @ CF[r_e] + W4 @ max_e x)

Distribution over 8 NeuronCores — "consumer computes" (no AllGather):
  - each core OWNS 64 edges and builds ONLY the <=128 distinct corner rows
    its edges reference: it dma_gathers the unique x rows incident to those
    corners (dedup'd, ~355 rows) and scatter-matmuls them against a
    host-built [pairs x corners] matrix (1/count folded in).  The local
    corner table round-trips through local DRAM (corner-major -> gather
    back channel-major), all at local HBM bandwidth — the only collective
    left is a 100 KB AllReduce(max) that hides under the phase-1 gathers.
  - conv stage: per 2-edge batch, 3 accumulating matmuls (x, left, right)
    with block-diagonal weights; the edge-independent G = W4 @ gmax term is
    hoisted, added on DVE, relu on ACT.
  - all data-plane tensors bf16 (inputs converted on host, output converted
    back); PSUM accumulation stays fp32.
"""

import os
import sys
import numpy as np
import ml_dtypes

for _p in ("/opt/trn_rl_repo", "/root/.axon_site/_ro/trn_rl_repo"):
    if os.path.isdir(_p) and _p not in sys.path:
        sys.path.insert(0, _p)

from concourse import bacc, bass, mybir, tile  # noqa: E402
from concourse.bass_utils import run_bass_kernel_spmd  # noqa: E402

BF16 = ml_dtypes.bfloat16

N_CORES = 8
E, C, H, W = 512, 64, 28, 28
HW = H * W                      # 784
HWP = 896                       # corner-table row pad: 896*2B = 1792B = 7*256
M_PAD = 128                     # local corner slots (64 edges * 2 <= 128)
E_LOC = E // N_CORES            # 64 edges per core
GROW = 6272                     # phase-1 gather elem: 8 channels * 784
NHALF = HW // 2                 # 392-wide matmul chunks

_PROGRAM_CACHE = {}


def _wrap_idxs(idx_flat, n_pad):
    """Pack flat gather indices into the dma_gather wrapped layout:
    [128, n_pad//16] int16 with logical index i at [i%16, i//16],
    replicated across the 8 groups of 16 partitions."""
    assert n_pad % 16 == 0
    w = np.zeros((16, n_pad // 16), dtype=np.int16)
    for i, v in enumerate(idx_flat):
        w[i % 16, i // 16] = v
    return np.tile(w, (8, 1))


def _prepare(x, W_agg, corner_edge_pairs, edge_corner, num_corners):
    x = np.asarray(x, dtype=np.float32)
    W_agg = np.asarray(W_agg, dtype=np.float32)
    cep = np.asarray(corner_edge_pairs).astype(np.int64)
    ec = np.asarray(edge_corner).astype(np.int64)
    ncorn = int(num_corners)
    assert x.shape == (E, C, H, W), x.shape

    # reference semantics: scatter drops out-of-range segments, gathers clamp
    seg = cep[:, 0]
    eid = np.clip(cep[:, 1], 0, E - 1)
    valid = (seg >= 0) & (seg < ncorn)
    seg_v, eid_v = seg[valid], eid[valid]
    ec_cl = np.clip(ec, 0, max(ncorn - 1, 0))

    counts = np.bincount(seg_v, minlength=max(ncorn, 1)).astype(np.int64)
    inv_count = 1.0 / np.maximum(counts, 1).astype(np.float64)

    xbf = x.reshape(E, C * HW).astype(BF16)
    xf = xbf.reshape(E * 8, GROW)               # 8-channel gather rows

    # block-diagonal weights for 2-edge batched conv matmuls
    wblk = np.zeros((4, 128, 128), dtype=BF16)
    for t in range(4):
        wt = W_agg[:, t * 64:(t + 1) * 64].T.astype(BF16)    # [c, o]
        wblk[t, :64, :64] = wt
        wblk[t, 64:, 64:] = wt
    wblk_in = wblk.reshape(512, 128)

    # cluster edges onto cores so each core's edge set shares corners:
    # fewer distinct corners => fewer unique incident x rows to gather and
    # fewer scatter-matmul K chunks.  greedy growth by min-new-corners.
    cedges = {}
    for cc, ee in zip(seg_v, eid_v):
        cedges.setdefault(int(cc), set()).add(int(ee))
    unassigned = set(range(E))
    groups = []
    for b in range(N_CORES):
        g = []
        gcorners = set()
        seed = min(unassigned)
        g.append(seed)
        unassigned.discard(seed)
        gcorners |= {int(ec_cl[seed][0]), int(ec_cl[seed][1])}
        while len(g) < E_LOC:
            cands = set()
            for cc in gcorners:
                cands |= (cedges.get(cc, set()) & unassigned)
            if not cands:
                cands = unassigned
            best, bestkey = None, None
            for e in cands:
                c1, c2 = int(ec_cl[e][0]), int(ec_cl[e][1])
                new = (c1 not in gcorners) + (c2 not in gcorners and c1 != c2)
                if bestkey is None or new < bestkey:
                    bestkey, best = new, e
                    if new == 0:
                        break
            g.append(best)
            unassigned.discard(best)
            gcorners |= {int(ec_cl[best][0]), int(ec_cl[best][1])}
        groups.append(np.array(g, dtype=np.int64))

    # per-core: distinct corners, unique incident edges, scatter matrix
    per_core_pre = []
    u_max = 0
    for b in range(N_CORES):
        edges_b = groups[b]
        corners = np.unique(ec_cl[edges_b])
        corners = corners[(corners >= 0) & (corners < max(ncorn, 1))]
        n_idx = {int(c): i for i, c in enumerate(corners)}
        assert len(corners) <= M_PAD
        pmask = np.isin(seg_v, corners)
        p_seg, p_eid = seg_v[pmask], eid_v[pmask]
        uniq = np.unique(p_eid)
        u_idx = {int(e): i for i, e in enumerate(uniq)}
        u_max = max(u_max, len(uniq))
        per_core_pre.append((corners, n_idx, p_seg, p_eid, u_idx, uniq))

    k_chunks = max(1, -(-u_max // 128))
    k_pad = 128 * k_chunks

    per_core = []
    for b in range(N_CORES):
        edges_b = groups[b]
        corners, n_idx, p_seg, p_eid, u_idx, uniq = per_core_pre[b]

        S = np.zeros((k_pad, M_PAD), dtype=np.float32)
        for cc, ee in zip(p_seg, p_eid):
            S[u_idx[int(ee)], n_idx[int(cc)]] += inv_count[cc]

        # stage-1 gather indices: per (kc, j): 128 idxs = eid*8 + j, padded
        # with edge 0 (real data; zero S rows nullify the contribution —
        # never pad with -1: skipped idxs leave stale SBUF and 0*NaN = NaN)
        s1_cols = []
        for kc in range(k_chunks):
            ids = np.zeros(128, dtype=np.int64)
            real = uniq[kc * 128:(kc + 1) * 128]
            ids[:len(real)] = real
            for j in range(8):
                s1_cols.append(_wrap_idxs((ids * 8 + j).astype(np.int16), 128))
        s1i = np.concatenate(s1_cols, axis=1)   # [128, k_chunks*64] int16

        # stage-4 gather indices: 8 instructions x 1024 idxs into the LOCAL
        # corner table: flat[s*128 + m*64 + c] = n_idx(corner)*64 + c
        s4_cols = []
        for binstr in range(8):
            flat = np.zeros(1024, dtype=np.int64)
            for ep in range(4):
                for t in range(2):
                    s = ep * 2 + t
                    for m in range(2):
                        el = m * 32 + binstr * 4 + ep
                        corner = int(ec_cl[edges_b[el], t])
                        base = n_idx.get(corner, 0) * 64
                        i0 = s * 128 + m * 64
                        flat[i0:i0 + 64] = base + np.arange(64)
            s4_cols.append(_wrap_idxs(flat.astype(np.int16), 1024))
        s4i = np.concatenate(s4_cols, axis=1)   # [128, 512] int16

        # local x, SBUF layout: [p = m*64+c, e32*784 + w]
        xl = (xbf[edges_b]
              .reshape(2, 32, C, HW)            # (m, e32, c, w)
              .transpose(0, 2, 1, 3)            # (m, c, e32, w)
              .reshape(128, 32 * HW))
        xl = np.ascontiguousarray(xl)

        per_core.append(dict(
            S=S.astype(BF16),
            s1i=s1i,
            s4i=s4i,
            xl=xl,
            edges=edges_b,
        ))

    return xf, wblk_in, per_core, k_chunks


# --------------------------------------------------------------------------
# device program
# --------------------------------------------------------------------------

def _build_program(k_chunks):
    bf = mybir.dt.bfloat16
    f32 = mybir.dt.float32
    i16 = mybir.dt.int16

    nc = bacc.Bacc("TRN2", target_bir_lowering=False, debug=False,
                   num_devices=N_CORES)

    xf_t = nc.dram_tensor("xf", [E * 8, GROW], bf, kind="ExternalInput").ap()
    xl_t = nc.dram_tensor("xl", [128, 32 * HW], bf, kind="ExternalInput").ap()
    wb_t = nc.dram_tensor("wb", [512, 128], bf, kind="ExternalInput").ap()
    sc_t = nc.dram_tensor("sc", [128 * k_chunks, M_PAD], bf, kind="ExternalInput").ap()
    s1_t = nc.dram_tensor("s1i", [128, k_chunks * 64], i16, kind="ExternalInput").ap()
    s4_t = nc.dram_tensor("s4i", [128, 512], i16, kind="ExternalInput").ap()
    out_t = nc.dram_tensor("out", [128, 32 * HW], bf, kind="ExternalOutput").ap()

    with tile.TileContext(nc) as tc:
        with tc.tile_pool(name="dram", bufs=1, space="DRAM") as dram, \
             tc.tile_pool(name="consts", bufs=1) as consts:
            cft = dram.tile([M_PAD * C, HWP], bf)         # local corner table
            gmx_in = dram.tile([64, HW], bf)
            gmx_out = dram.tile([64, HW], bf, addr_space="Shared")

            # constants (gather indices first so phase-1 gathers start ASAP)
            s1tile = consts.tile([128, k_chunks * 64], i16)
            nc.gpsimd.dma_start(out=s1tile[:], in_=s1_t[:])
            sctiles = []
            for kc in range(k_chunks):
                st = consts.tile([128, M_PAD], bf, tag=f"sc{kc}")
                nc.gpsimd.dma_start(out=st[:], in_=sc_t[kc * 128:(kc + 1) * 128, :])
                sctiles.append(st)
            s4tile = consts.tile([128, 512], i16)
            nc.sync.dma_start(out=s4tile[:], in_=s4_t[:])
            wtiles = []
            for t in range(4):
                wt = consts.tile([128, 128], bf, tag=f"w{t}")
                nc.sync.dma_start(out=wt[:], in_=wb_t[t * 128:(t + 1) * 128, :])
                wtiles.append(wt)
            gm2 = consts.tile([128, HW], bf, tag="gm2")
            gsb = consts.tile([128, HW], f32, tag="gsb")

            # local x, kept in SBUF through phase 4 (loaded AFTER the
            # phase-1 gather-gens below: its 6.4MB would otherwise occupy the
            # DMA engines and delay the tiny s1 index load that gates them)
            xkeep = consts.tile([128, 32 * HW], bf, tag="xkeep")

            # ---------------- phase 1: build local corner table -----------
            with tc.tile_pool(name="p1", bufs=min(3 * k_chunks + 1, 7)) as p1, \
                 tc.tile_pool(name="p1s", bufs=2) as p1s, \
                 tc.tile_pool(name="p2", bufs=1) as p2, \
                 tc.tile_pool(name="psum1", bufs=8, space="PSUM") as psum1:
                # PE warmup: the HAM clock governor sits at K=4 (1.2 GHz)
                # through the DMA-bound start; ~56 dummy matmuls on garbage
                # bits escalate it to K=8 before the real MM stream begins.
                # Numerics are irrelevant; the last tile is sunk to DRAM so
                # the chain is not dead-code-eliminated.
                wsink = dram.tile([128, NHALF], f32)
                pwlast = None
                for wi in range(56):
                    pw = psum1.tile([128, NHALF], f32, space="PSUM", tag="ps1",
                                    name=f"pw{wi}")
                    nc.tensor.matmul(out=pw[:], lhsT=wtiles[0][:],
                                     rhs=s4tile[:, 0:NHALF].bitcast(bf),
                                     start=True, stop=True)
                    pwlast = pw
                wsb = p2.tile([128, NHALF], f32, tag="wsb")
                nc.vector.tensor_copy(out=wsb[:], in_=pwlast[:])
                nc.sync.dma_start(out=wsink[:], in_=wsb[:])

                gts = {}
                for j in range(8):
                    for kc in range(k_chunks):
                        gt = p1.tile([128, GROW], bf, tag="gt",
                                     name=f"gt_{kc}_{j}")
                        nc.gpsimd.dma_gather(
                            gt[:].rearrange("p (s d) -> p s d", d=GROW),
                            xf_t[:],
                            s1tile[:, (kc * 8 + j) * 8:(kc * 8 + j) * 8 + 8],
                            num_idxs=128, num_idxs_reg=128,
                            elem_size=GROW,
                        )
                        gts[(kc, j)] = gt

                nc.scalar.dma_start(out=xkeep[:], in_=xl_t[:])

                # phase 2: local max tree on DVE (emitted after the xkeep
                # load it reads), then the AllReduce trigger.  The collective
                # instruction blocks the GpSimd queue on the gmx semaphore,
                # so it must come after all gather descriptor-gens.
                scr = p2.tile([128, 16 * HW], bf, tag="scr")
                nc.vector.tensor_tensor(
                    out=scr[:], in0=xkeep[:, 0:16 * HW],
                    in1=xkeep[:, 16 * HW:32 * HW],
                    op=mybir.AluOpType.max)
                wdt = 8 * HW
                while wdt >= HW:
                    nc.vector.tensor_tensor(
                        out=scr[:, 0:wdt], in0=scr[:, 0:wdt],
                        in1=scr[:, wdt:2 * wdt],
                        op=mybir.AluOpType.max)
                    wdt //= 2
                halfm = p2.tile([64, HW], bf, tag="halfm")
                nc.sync.dma_start(out=halfm[:], in_=scr[64:128, 0:HW])
                nc.vector.tensor_tensor(
                    out=scr[0:64, 0:HW], in0=scr[0:64, 0:HW],
                    in1=halfm[:], op=mybir.AluOpType.max)
                nc.sync.dma_start(out=gmx_in[:], in_=scr[0:64, 0:HW])

                nc.gpsimd.collective_compute(
                    "AllReduce", mybir.AluOpType.max,
                    replica_groups=[list(range(N_CORES))],
                    ins=[gmx_in.opt()], outs=[gmx_out.opt()],
                )

                for j in range(8):
                    # scatter matmuls for channel group j: full 128-corner M.
                    # kc outer within each 4-chunk group => runs of 4
                    # consecutive matmuls share the same stationary weights
                    stg = p1s.tile([128, 8 * HWP], bf, tag="stg")
                    for ng in range(4):
                        pss = [psum1.tile([128, NHALF], f32, space="PSUM",
                                          tag="ps1", name=f"ps1_{j}_{ng}_{i}")
                               for i in range(4)]
                        for kc in range(k_chunks):
                            for n4 in range(4):
                                n = ng * 4 + n4
                                nc.tensor.matmul(
                                    out=pss[n4][:],
                                    lhsT=sctiles[kc][:],
                                    rhs=gts[(kc, j)][:, n * NHALF:(n + 1) * NHALF],
                                    start=(kc == 0), stop=(kc == k_chunks - 1),
                                )
                        for n4 in range(4):
                            n = ng * 4 + n4
                            c8, half = n // 2, n % 2
                            dst = stg[:, c8 * HWP + half * NHALF:
                                      c8 * HWP + half * NHALF + NHALF]
                            # alternate cast engine so neither DVE nor ACT paces
                            if n % 2 == 0:
                                nc.vector.tensor_copy(out=dst, in_=pss[n4][:])
                            else:
                                nc.scalar.activation(
                                    dst, pss[n4][:],
                                    mybir.ActivationFunctionType.Copy)
                    # staging -> local table rows n*64 + j*8 + c8
                    nc.sync.dma_start(
                        out=cft[:].rearrange(
                            "(m j c8) w -> j m (c8 w)", m=M_PAD, j=8, c8=8)[j],
                        in_=stg[:],
                    )

                for wi in range(48):
                    pw2 = psum1.tile([128, NHALF], f32, space="PSUM", tag="ps1",
                                     name=f"pw2_{wi}")
                    nc.tensor.matmul(out=pw2[:], lhsT=wtiles[0][:],
                                     rhs=s4tile[:, 0:NHALF].bitcast(bf),
                                     start=True, stop=True)
                    pwlast = pw2
                wsb2 = p2.tile([128, NHALF], f32, tag="wsb2")
                nc.vector.tensor_copy(out=wsb2[:], in_=pwlast[:])
                nc.sync.dma_start(out=wsink[:], in_=wsb2[:])

            # global max -> gm2 both halves, then G = W4blk @ gmax
            with tc.tile_pool(name="psumg", bufs=2, space="PSUM") as psumg:
                nc.sync.dma_start(out=gm2[0:64, :], in_=gmx_out[:])
                nc.sync.dma_start(out=gm2[64:128, :], in_=gmx_out[:])
                for hh in range(2):
                    sl = slice(hh * NHALF, (hh + 1) * NHALF)
                    psg = psumg.tile([128, NHALF], f32, space="PSUM", tag="psg")
                    nc.tensor.matmul(out=psg[:], lhsT=wtiles[3][:],
                                     rhs=gm2[:, sl], start=True, stop=True)
                    nc.vector.tensor_copy(out=gsb[:, sl], in_=psg[:])

            # ---------------- phase 4: conv -------------------------------
            with tc.tile_pool(name="p4", bufs=4) as p4, \
                 tc.tile_pool(name="p4o", bufs=3) as p4o, \
                 tc.tile_pool(name="p4t", bufs=4) as p4t, \
                 tc.tile_pool(name="psum4", bufs=8, space="PSUM") as psum4:
                for binstr in range(8):
                    lrt = p4.tile([128, 8 * HWP], bf, tag="lrt")
                    nc.gpsimd.dma_gather(
                        lrt[:].rearrange("p (s d) -> p s d", d=HWP),
                        cft[:],
                        s4tile[:, binstr * 64:(binstr + 1) * 64],
                        num_idxs=1024, num_idxs_reg=1024, elem_size=HWP,
                    )
                    # weight-grouped matmul order: runs of 8 share lhsT
                    pss = {}
                    for ep in range(4):
                        for hh in range(2):
                            pss[(ep, hh)] = psum4.tile(
                                [128, NHALF], f32, space="PSUM", tag="ps4",
                                name=f"ps4_{binstr}_{ep}_{hh}")
                    for ep in range(4):
                        e32 = binstr * 4 + ep
                        for hh in range(2):
                            nc.tensor.matmul(out=pss[(ep, hh)][:], lhsT=wtiles[0][:],
                                             rhs=xkeep[:, e32 * HW + hh * NHALF:
                                                       e32 * HW + hh * NHALF + NHALF],
                                             start=True, stop=False)
                    for ep in range(4):
                        for hh in range(2):
                            nc.tensor.matmul(out=pss[(ep, hh)][:], lhsT=wtiles[1][:],
                                             rhs=lrt[:, (ep * 2) * HWP + hh * NHALF:
                                                     (ep * 2) * HWP + hh * NHALF + NHALF],
                                             start=False, stop=False)
                    for ep in range(4):
                        for hh in range(2):
                            nc.tensor.matmul(out=pss[(ep, hh)][:], lhsT=wtiles[2][:],
                                             rhs=lrt[:, (ep * 2 + 1) * HWP + hh * NHALF:
                                                     (ep * 2 + 1) * HWP + hh * NHALF + NHALF],
                                             start=False, stop=True)
                    for ep in range(4):
                        e32 = binstr * 4 + ep
                        ot = p4o.tile([128, HW], bf, tag="ot")
                        for hh in range(2):
                            sl = slice(hh * NHALF, (hh + 1) * NHALF)
                            # add the hoisted gmax term on DVE, relu on ACT
                            tmp = p4t.tile([128, NHALF], bf, tag="tmp")
                            nc.vector.tensor_tensor(out=tmp[:], in0=pss[(ep, hh)][:],
                                                    in1=gsb[:, sl],
                                                    op=mybir.AluOpType.add)
                            nc.scalar.activation(ot[:, sl], tmp[:],
                                                 mybir.ActivationFunctionType.Relu)
                        nc.sync.dma_start(
                            out=out_t[:, e32 * HW:(e32 + 1) * HW],
                            in_=ot[:],
                        )

    nc.compile()
    return nc


# --------------------------------------------------------------------------
# entry point
# --------------------------------------------------------------------------

def _unpack_out(ob):
    """[128, 32*784] bf16 (p = m*64+c, e32*784+w) -> [64, 64, 28, 28] f32."""
    o = np.asarray(ob, dtype=np.float32).reshape(2, C, 32, HW)
    return o.transpose(0, 2, 1, 3).reshape(E_LOC, C, H, W)


def _run(x, W_agg, corner_edge_pairs, edge_corner, num_corners,
         trace=False):
    xf, wblk_in, per_core, k_chunks = _prepare(
        x, W_agg, corner_edge_pairs, edge_corner, num_corners)

    if k_chunks not in _PROGRAM_CACHE:
        _PROGRAM_CACHE[k_chunks] = _build_program(k_chunks)
    nc = _PROGRAM_CACHE[k_chunks]

    in_maps = [{
        "xf": xf, "xl": pc["xl"], "wb": wblk_in,
        "sc": pc["S"], "s1i": pc["s1i"], "s4i": pc["s4i"],
    } for pc in per_core]

    kwargs = dict(trace=trace)
    if trace:
        kwargs["trace_cores"] = list(range(N_CORES))
    res = run_bass_kernel_spmd(nc, in_maps, list(range(N_CORES)), **kwargs)

    out = np.empty((E, C, H, W), dtype=np.float32)
    for b in range(N_CORES):
        out[per_core[b]["edges"]] = _unpack_out(res.results[b]["out"])
    return out, res


def kernel(x, W_agg, corner_edge_pairs, edge_corner, num_corners):
    out, _ = _run(x, W_agg, corner_edge_pairs, edge_corner, num_corners,
                  trace=False)
    return out


# expose for test harness profiling
def _run_profiled(x, W_agg, corner_edge_pairs, edge_corner, num_corners,
                  trace=True):
    return _run(x, W_agg, corner_edge_pairs, edge_corner, num_corners,
                trace=trace)


# revision 35
# speedup vs baseline: 1.0196x; 1.0196x over previous
"""Trainium2 Bass kernel for the gnn_message_passing LoopModel.

Reference computation (per edge e, corners l/r from edge_corner):
    CF[n]    = mean over pairs (n, e') of x[e']          (segment mean)
    out[e]   = relu(W1 @ x[e] + W2 @ CF[l_e] + W3 @ CF[r_e] + W4 @ max_e x)

Distribution over 8 NeuronCores — "consumer computes" (no AllGather):
  - each core OWNS 64 edges and builds ONLY the <=128 distinct corner rows
    its edges reference: it dma_gathers the unique x rows incident to those
    corners (dedup'd, ~355 rows) and scatter-matmuls them against a
    host-built [pairs x corners] matrix (1/count folded in).  The local
    corner table round-trips through local DRAM (corner-major -> gather
    back channel-major), all at local HBM bandwidth — the only collective
    left is a 100 KB AllReduce(max) that hides under the phase-1 gathers.
  - conv stage: per 2-edge batch, 3 accumulating matmuls (x, left, right)
    with block-diagonal weights; the edge-independent G = W4 @ gmax term is
    hoisted, added on DVE, relu on ACT.
  - all data-plane tensors bf16 (inputs converted on host, output converted
    back); PSUM accumulation stays fp32.
"""

import os
import sys
import numpy as np
import ml_dtypes

for _p in ("/opt/trn_rl_repo", "/root/.axon_site/_ro/trn_rl_repo"):
    if os.path.isdir(_p) and _p not in sys.path:
        sys.path.insert(0, _p)

from concourse import bacc, bass, mybir, tile  # noqa: E402
from concourse.bass_utils import run_bass_kernel_spmd  # noqa: E402

BF16 = ml_dtypes.bfloat16

N_CORES = 8
E, C, H, W = 512, 64, 28, 28
HW = H * W                      # 784
HWP = 896                       # corner-table row pad: 896*2B = 1792B = 7*256
M_PAD = 128                     # local corner slots (64 edges * 2 <= 128)
E_LOC = E // N_CORES            # 64 edges per core
GROW = 6272                     # phase-1 gather elem: 8 channels * 784
NHALF = HW // 2                 # 392-wide matmul chunks

_PROGRAM_CACHE = {}


def _wrap_idxs(idx_flat, n_pad):
    """Pack flat gather indices into the dma_gather wrapped layout:
    [128, n_pad//16] int16 with logical index i at [i%16, i//16],
    replicated across the 8 groups of 16 partitions."""
    assert n_pad % 16 == 0
    w = np.zeros((16, n_pad // 16), dtype=np.int16)
    for i, v in enumerate(idx_flat):
        w[i % 16, i // 16] = v
    return np.tile(w, (8, 1))


def _prepare(x, W_agg, corner_edge_pairs, edge_corner, num_corners):
    x = np.asarray(x, dtype=np.float32)
    W_agg = np.asarray(W_agg, dtype=np.float32)
    cep = np.asarray(corner_edge_pairs).astype(np.int64)
    ec = np.asarray(edge_corner).astype(np.int64)
    ncorn = int(num_corners)
    assert x.shape == (E, C, H, W), x.shape

    # reference semantics: scatter drops out-of-range segments, gathers clamp
    seg = cep[:, 0]
    eid = np.clip(cep[:, 1], 0, E - 1)
    valid = (seg >= 0) & (seg < ncorn)
    seg_v, eid_v = seg[valid], eid[valid]
    ec_cl = np.clip(ec, 0, max(ncorn - 1, 0))

    counts = np.bincount(seg_v, minlength=max(ncorn, 1)).astype(np.int64)
    inv_count = 1.0 / np.maximum(counts, 1).astype(np.float64)

    xbf = x.reshape(E, C * HW).astype(BF16)
    xf = xbf.reshape(E * 8, GROW)               # 8-channel gather rows

    # block-diagonal weights for 2-edge batched conv matmuls
    wblk = np.zeros((4, 128, 128), dtype=BF16)
    for t in range(4):
        wt = W_agg[:, t * 64:(t + 1) * 64].T.astype(BF16)    # [c, o]
        wblk[t, :64, :64] = wt
        wblk[t, 64:, 64:] = wt
    wblk_in = wblk.reshape(512, 128)

    # cluster edges onto cores so each core's edge set shares corners:
    # fewer distinct corners => fewer unique incident x rows to gather and
    # fewer scatter-matmul K chunks.  greedy growth by min-new-corners.
    cedges = {}
    for cc, ee in zip(seg_v, eid_v):
        cedges.setdefault(int(cc), set()).add(int(ee))
    unassigned = set(range(E))
    groups = []
    for b in range(N_CORES):
        g = []
        gcorners = set()
        seed = min(unassigned)
        g.append(seed)
        unassigned.discard(seed)
        gcorners |= {int(ec_cl[seed][0]), int(ec_cl[seed][1])}
        while len(g) < E_LOC:
            cands = set()
            for cc in gcorners:
                cands |= (cedges.get(cc, set()) & unassigned)
            if not cands:
                cands = unassigned
            best, bestkey = None, None
            for e in cands:
                c1, c2 = int(ec_cl[e][0]), int(ec_cl[e][1])
                new = (c1 not in gcorners) + (c2 not in gcorners and c1 != c2)
                if bestkey is None or new < bestkey:
                    bestkey, best = new, e
                    if new == 0:
                        break
            g.append(best)
            unassigned.discard(best)
            gcorners |= {int(ec_cl[best][0]), int(ec_cl[best][1])}
        groups.append(np.array(g, dtype=np.int64))

    # per-core: distinct corners, unique incident edges, scatter matrix
    per_core_pre = []
    u_max = 0
    for b in range(N_CORES):
        edges_b = groups[b]
        corners = np.unique(ec_cl[edges_b])
        corners = corners[(corners >= 0) & (corners < max(ncorn, 1))]
        n_idx = {int(c): i for i, c in enumerate(corners)}
        assert len(corners) <= M_PAD
        pmask = np.isin(seg_v, corners)
        p_seg, p_eid = seg_v[pmask], eid_v[pmask]
        uniq = np.unique(p_eid)
        u_idx = {int(e): i for i, e in enumerate(uniq)}
        u_max = max(u_max, len(uniq))
        per_core_pre.append((corners, n_idx, p_seg, p_eid, u_idx, uniq))

    k_chunks = max(1, -(-u_max // 128))
    k_pad = 128 * k_chunks

    per_core = []
    for b in range(N_CORES):
        edges_b = groups[b]
        corners, n_idx, p_seg, p_eid, u_idx, uniq = per_core_pre[b]

        S = np.zeros((k_pad, M_PAD), dtype=np.float32)
        for cc, ee in zip(p_seg, p_eid):
            S[u_idx[int(ee)], n_idx[int(cc)]] += inv_count[cc]

        # stage-1 gather indices: per (kc, j): 128 idxs = eid*8 + j, padded
        # with edge 0 (real data; zero S rows nullify the contribution —
        # never pad with -1: skipped idxs leave stale SBUF and 0*NaN = NaN)
        s1_cols = []
        for kc in range(k_chunks):
            ids = np.zeros(128, dtype=np.int64)
            real = uniq[kc * 128:(kc + 1) * 128]
            ids[:len(real)] = real
            for j in range(8):
                s1_cols.append(_wrap_idxs((ids * 8 + j).astype(np.int16), 128))
        s1i = np.concatenate(s1_cols, axis=1)   # [128, k_chunks*64] int16

        # stage-4 gather indices: 8 instructions x 1024 idxs into the LOCAL
        # corner table: flat[s*128 + m*64 + c] = n_idx(corner)*64 + c
        s4_cols = []
        for binstr in range(8):
            flat = np.zeros(1024, dtype=np.int64)
            for ep in range(4):
                for t in range(2):
                    s = ep * 2 + t
                    for m in range(2):
                        el = m * 32 + binstr * 4 + ep
                        corner = int(ec_cl[edges_b[el], t])
                        base = n_idx.get(corner, 0) * 64
                        i0 = s * 128 + m * 64
                        flat[i0:i0 + 64] = base + np.arange(64)
            s4_cols.append(_wrap_idxs(flat.astype(np.int16), 1024))
        s4i = np.concatenate(s4_cols, axis=1)   # [128, 512] int16

        # local x, SBUF layout: [p = m*64+c, e32*784 + w]
        xl = (xbf[edges_b]
              .reshape(2, 32, C, HW)            # (m, e32, c, w)
              .transpose(0, 2, 1, 3)            # (m, c, e32, w)
              .reshape(128, 32 * HW))
        xl = np.ascontiguousarray(xl)

        per_core.append(dict(
            S=S.astype(BF16),
            s1i=s1i,
            s4i=s4i,
            xl=xl,
            edges=edges_b,
        ))

    return xf, wblk_in, per_core, k_chunks


# --------------------------------------------------------------------------
# device program
# --------------------------------------------------------------------------

def _build_program(k_chunks):
    bf = mybir.dt.bfloat16
    f32 = mybir.dt.float32
    i16 = mybir.dt.int16

    nc = bacc.Bacc("TRN2", target_bir_lowering=False, debug=False,
                   num_devices=N_CORES)

    xf_t = nc.dram_tensor("xf", [E * 8, GROW], bf, kind="ExternalInput").ap()
    xl_t = nc.dram_tensor("xl", [128, 32 * HW], bf, kind="ExternalInput").ap()
    wb_t = nc.dram_tensor("wb", [512, 128], bf, kind="ExternalInput").ap()
    sc_t = nc.dram_tensor("sc", [128 * k_chunks, M_PAD], bf, kind="ExternalInput").ap()
    s1_t = nc.dram_tensor("s1i", [128, k_chunks * 64], i16, kind="ExternalInput").ap()
    s4_t = nc.dram_tensor("s4i", [128, 512], i16, kind="ExternalInput").ap()
    out_t = nc.dram_tensor("out", [128, 32 * HW], bf, kind="ExternalOutput").ap()

    with tile.TileContext(nc) as tc:
        with tc.tile_pool(name="dram", bufs=1, space="DRAM") as dram, \
             tc.tile_pool(name="consts", bufs=1) as consts:
            cft = dram.tile([M_PAD * C, HWP], bf)         # local corner table
            gmx_in = dram.tile([64, HW], bf)
            gmx_out = dram.tile([64, HW], bf, addr_space="Shared")

            # constants (gather indices first so phase-1 gathers start ASAP)
            s1tile = consts.tile([128, k_chunks * 64], i16)
            nc.gpsimd.dma_start(out=s1tile[:], in_=s1_t[:])
            sctiles = []
            for kc in range(k_chunks):
                st = consts.tile([128, M_PAD], bf, tag=f"sc{kc}")
                nc.gpsimd.dma_start(out=st[:], in_=sc_t[kc * 128:(kc + 1) * 128, :])
                sctiles.append(st)
            s4tile = consts.tile([128, 512], i16)
            nc.sync.dma_start(out=s4tile[:], in_=s4_t[:])
            wtiles = []
            for t in range(4):
                wt = consts.tile([128, 128], bf, tag=f"w{t}")
                nc.sync.dma_start(out=wt[:], in_=wb_t[t * 128:(t + 1) * 128, :])
                wtiles.append(wt)
            gm2 = consts.tile([128, HW], bf, tag="gm2")
            gsb = consts.tile([128, HW], f32, tag="gsb")

            # local x, kept in SBUF through phase 4 (loaded AFTER the
            # phase-1 gather-gens below: its 6.4MB would otherwise occupy the
            # DMA engines and delay the tiny s1 index load that gates them)
            xkeep = consts.tile([128, 32 * HW], bf, tag="xkeep")

            # ---------------- phase 1: build local corner table -----------
            with tc.tile_pool(name="p1", bufs=min(3 * k_chunks + 1, 7)) as p1, \
                 tc.tile_pool(name="p1s", bufs=2) as p1s, \
                 tc.tile_pool(name="p2", bufs=1) as p2, \
                 tc.tile_pool(name="psum1", bufs=8, space="PSUM") as psum1:
                # PE warmup: the HAM clock governor sits at K=4 (1.2 GHz)
                # through the DMA-bound start; ~56 dummy matmuls on garbage
                # bits escalate it to K=8 before the real MM stream begins.
                # Numerics are irrelevant; the last tile is sunk to DRAM so
                # the chain is not dead-code-eliminated.
                wsink = dram.tile([128, NHALF], f32)
                pwlast = None
                for wi in range(56):
                    pw = psum1.tile([128, NHALF], f32, space="PSUM", tag="ps1",
                                    name=f"pw{wi}")
                    nc.tensor.matmul(out=pw[:, 0:M_PAD], lhsT=sctiles[0][:],
                                     rhs=sctiles[k_chunks - 1][:],
                                     start=True, stop=True)
                    pwlast = pw
                wsb = p2.tile([128, NHALF], f32, tag="wsb")
                nc.vector.tensor_copy(out=wsb[:], in_=pwlast[:])
                nc.sync.dma_start(out=wsink[:], in_=wsb[:])

                gts = {}
                for j in range(8):
                    for kc in range(k_chunks):
                        gt = p1.tile([128, GROW], bf, tag="gt",
                                     name=f"gt_{kc}_{j}")
                        nc.gpsimd.dma_gather(
                            gt[:].rearrange("p (s d) -> p s d", d=GROW),
                            xf_t[:],
                            s1tile[:, (kc * 8 + j) * 8:(kc * 8 + j) * 8 + 8],
                            num_idxs=128, num_idxs_reg=128,
                            elem_size=GROW,
                        )
                        gts[(kc, j)] = gt

                nc.scalar.dma_start(out=xkeep[:], in_=xl_t[:])

                # phase 2: local max tree on DVE (emitted after the xkeep
                # load it reads), then the AllReduce trigger.  The collective
                # instruction blocks the GpSimd queue on the gmx semaphore,
                # so it must come after all gather descriptor-gens.
                scr = p2.tile([128, 16 * HW], bf, tag="scr")
                nc.vector.tensor_tensor(
                    out=scr[:], in0=xkeep[:, 0:16 * HW],
                    in1=xkeep[:, 16 * HW:32 * HW],
                    op=mybir.AluOpType.max)
                wdt = 8 * HW
                while wdt >= HW:
                    nc.vector.tensor_tensor(
                        out=scr[:, 0:wdt], in0=scr[:, 0:wdt],
                        in1=scr[:, wdt:2 * wdt],
                        op=mybir.AluOpType.max)
                    wdt //= 2
                halfm = p2.tile([64, HW], bf, tag="halfm")
                nc.sync.dma_start(out=halfm[:], in_=scr[64:128, 0:HW])
                nc.vector.tensor_tensor(
                    out=scr[0:64, 0:HW], in0=scr[0:64, 0:HW],
                    in1=halfm[:], op=mybir.AluOpType.max)
                nc.sync.dma_start(out=gmx_in[:], in_=scr[0:64, 0:HW])

                nc.gpsimd.collective_compute(
                    "AllReduce", mybir.AluOpType.max,
                    replica_groups=[list(range(N_CORES))],
                    ins=[gmx_in.opt()], outs=[gmx_out.opt()],
                )

                for j in range(8):
                    # scatter matmuls for channel group j: full 128-corner M.
                    # kc outer within each 4-chunk group => runs of 4
                    # consecutive matmuls share the same stationary weights
                    stg = p1s.tile([128, 8 * HWP], bf, tag="stg")
                    for ng in range(4):
                        pss = [psum1.tile([128, NHALF], f32, space="PSUM",
                                          tag="ps1", name=f"ps1_{j}_{ng}_{i}")
                               for i in range(4)]
                        for kc in range(k_chunks):
                            for n4 in range(4):
                                n = ng * 4 + n4
                                nc.tensor.matmul(
                                    out=pss[n4][:],
                                    lhsT=sctiles[kc][:],
                                    rhs=gts[(kc, j)][:, n * NHALF:(n + 1) * NHALF],
                                    start=(kc == 0), stop=(kc == k_chunks - 1),
                                )
                        for n4 in range(4):
                            n = ng * 4 + n4
                            c8, half = n // 2, n % 2
                            dst = stg[:, c8 * HWP + half * NHALF:
                                      c8 * HWP + half * NHALF + NHALF]
                            # alternate cast engine so neither DVE nor ACT paces
                            if n % 2 == 0:
                                nc.vector.tensor_copy(out=dst, in_=pss[n4][:])
                            else:
                                nc.scalar.activation(
                                    dst, pss[n4][:],
                                    mybir.ActivationFunctionType.Copy)
                    # staging -> local table rows n*64 + j*8 + c8
                    nc.sync.dma_start(
                        out=cft[:].rearrange(
                            "(m j c8) w -> j m (c8 w)", m=M_PAD, j=8, c8=8)[j],
                        in_=stg[:],
                    )

                for wi in range(48):
                    pw2 = psum1.tile([128, NHALF], f32, space="PSUM", tag="ps1",
                                     name=f"pw2_{wi}")
                    nc.tensor.matmul(out=pw2[:], lhsT=wtiles[0][:],
                                     rhs=s4tile[:, 0:NHALF].bitcast(bf),
                                     start=True, stop=True)
                    pwlast = pw2
                wsb2 = p2.tile([128, NHALF], f32, tag="wsb2")
                nc.vector.tensor_copy(out=wsb2[:], in_=pwlast[:])
                nc.sync.dma_start(out=wsink[:], in_=wsb2[:])

            # global max -> gm2 both halves, then G = W4blk @ gmax
            with tc.tile_pool(name="psumg", bufs=2, space="PSUM") as psumg:
                nc.sync.dma_start(out=gm2[0:64, :], in_=gmx_out[:])
                nc.sync.dma_start(out=gm2[64:128, :], in_=gmx_out[:])
                for hh in range(2):
                    sl = slice(hh * NHALF, (hh + 1) * NHALF)
                    psg = psumg.tile([128, NHALF], f32, space="PSUM", tag="psg")
                    nc.tensor.matmul(out=psg[:], lhsT=wtiles[3][:],
                                     rhs=gm2[:, sl], start=True, stop=True)
                    nc.vector.tensor_copy(out=gsb[:, sl], in_=psg[:])

            # ---------------- phase 4: conv -------------------------------
            with tc.tile_pool(name="p4", bufs=4) as p4, \
                 tc.tile_pool(name="p4o", bufs=3) as p4o, \
                 tc.tile_pool(name="p4t", bufs=4) as p4t, \
                 tc.tile_pool(name="psum4", bufs=8, space="PSUM") as psum4:
                for binstr in range(8):
                    lrt = p4.tile([128, 8 * HWP], bf, tag="lrt")
                    nc.gpsimd.dma_gather(
                        lrt[:].rearrange("p (s d) -> p s d", d=HWP),
                        cft[:],
                        s4tile[:, binstr * 64:(binstr + 1) * 64],
                        num_idxs=1024, num_idxs_reg=1024, elem_size=HWP,
                    )
                    # weight-grouped matmul order: runs of 8 share lhsT
                    pss = {}
                    for ep in range(4):
                        for hh in range(2):
                            pss[(ep, hh)] = psum4.tile(
                                [128, NHALF], f32, space="PSUM", tag="ps4",
                                name=f"ps4_{binstr}_{ep}_{hh}")
                    for ep in range(4):
                        e32 = binstr * 4 + ep
                        for hh in range(2):
                            nc.tensor.matmul(out=pss[(ep, hh)][:], lhsT=wtiles[0][:],
                                             rhs=xkeep[:, e32 * HW + hh * NHALF:
                                                       e32 * HW + hh * NHALF + NHALF],
                                             start=True, stop=False)
                    for ep in range(4):
                        for hh in range(2):
                            nc.tensor.matmul(out=pss[(ep, hh)][:], lhsT=wtiles[1][:],
                                             rhs=lrt[:, (ep * 2) * HWP + hh * NHALF:
                                                     (ep * 2) * HWP + hh * NHALF + NHALF],
                                             start=False, stop=False)
                    for ep in range(4):
                        for hh in range(2):
                            nc.tensor.matmul(out=pss[(ep, hh)][:], lhsT=wtiles[2][:],
                                             rhs=lrt[:, (ep * 2 + 1) * HWP + hh * NHALF:
                                                     (ep * 2 + 1) * HWP + hh * NHALF + NHALF],
                                             start=False, stop=True)
                    for ep in range(4):
                        e32 = binstr * 4 + ep
                        ot = p4o.tile([128, HW], bf, tag="ot")
                        for hh in range(2):
                            sl = slice(hh * NHALF, (hh + 1) * NHALF)
                            # add the hoisted gmax term on DVE, relu on ACT
                            tmp = p4t.tile([128, NHALF], bf, tag="tmp")
                            nc.vector.tensor_tensor(out=tmp[:], in0=pss[(ep, hh)][:],
                                                    in1=gsb[:, sl],
                                                    op=mybir.AluOpType.add)
                            nc.scalar.activation(ot[:, sl], tmp[:],
                                                 mybir.ActivationFunctionType.Relu)
                        nc.sync.dma_start(
                            out=out_t[:, e32 * HW:(e32 + 1) * HW],
                            in_=ot[:],
                        )

    nc.compile()
    return nc


# --------------------------------------------------------------------------
# entry point
# --------------------------------------------------------------------------

def _unpack_out(ob):
    """[128, 32*784] bf16 (p = m*64+c, e32*784+w) -> [64, 64, 28, 28] f32."""
    o = np.asarray(ob, dtype=np.float32).reshape(2, C, 32, HW)
    return o.transpose(0, 2, 1, 3).reshape(E_LOC, C, H, W)


def _run(x, W_agg, corner_edge_pairs, edge_corner, num_corners,
         trace=False):
    xf, wblk_in, per_core, k_chunks = _prepare(
        x, W_agg, corner_edge_pairs, edge_corner, num_corners)

    if k_chunks not in _PROGRAM_CACHE:
        _PROGRAM_CACHE[k_chunks] = _build_program(k_chunks)
    nc = _PROGRAM_CACHE[k_chunks]

    in_maps = [{
        "xf": xf, "xl": pc["xl"], "wb": wblk_in,
        "sc": pc["S"], "s1i": pc["s1i"], "s4i": pc["s4i"],
    } for pc in per_core]

    kwargs = dict(trace=trace)
    if trace:
        kwargs["trace_cores"] = list(range(N_CORES))
    res = run_bass_kernel_spmd(nc, in_maps, list(range(N_CORES)), **kwargs)

    out = np.empty((E, C, H, W), dtype=np.float32)
    for b in range(N_CORES):
        out[per_core[b]["edges"]] = _unpack_out(res.results[b]["out"])
    return out, res


def kernel(x, W_agg, corner_edge_pairs, edge_corner, num_corners):
    out, _ = _run(x, W_agg, corner_edge_pairs, edge_corner, num_corners,
                  trace=False)
    return out


# expose for test harness profiling
def _run_profiled(x, W_agg, corner_edge_pairs, edge_corner, num_corners,
                  trace=True):
    return _run(x, W_agg, corner_edge_pairs, edge_corner, num_corners,
                trace=trace)


# revision 36
# speedup vs baseline: 1.0595x; 1.0391x over previous
"""Trainium2 Bass kernel for the gnn_message_passing LoopModel.

Reference computation (per edge e, corners l/r from edge_corner):
    CF[n]    = mean over pairs (n, e') of x[e']          (segment mean)
    out[e]   = relu(W1 @ x[e] + W2 @ CF[l_e] + W3 @ CF[r_e] + W4 @ max_e x)

Distribution over 8 NeuronCores — "consumer computes" (no AllGather):
  - each core OWNS 64 edges and builds ONLY the <=128 distinct corner rows
    its edges reference: it dma_gathers the unique x rows incident to those
    corners (dedup'd, ~355 rows) and scatter-matmuls them against a
    host-built [pairs x corners] matrix (1/count folded in).  The local
    corner table round-trips through local DRAM (corner-major -> gather
    back channel-major), all at local HBM bandwidth — the only collective
    left is a 100 KB AllReduce(max) that hides under the phase-1 gathers.
  - conv stage: per 2-edge batch, 3 accumulating matmuls (x, left, right)
    with block-diagonal weights; the edge-independent G = W4 @ gmax term is
    hoisted, added on DVE, relu on ACT.
  - all data-plane tensors bf16 (inputs converted on host, output converted
    back); PSUM accumulation stays fp32.
"""

import os
import sys
import numpy as np
import ml_dtypes

for _p in ("/opt/trn_rl_repo", "/root/.axon_site/_ro/trn_rl_repo"):
    if os.path.isdir(_p) and _p not in sys.path:
        sys.path.insert(0, _p)

from concourse import bacc, bass, mybir, tile  # noqa: E402
from concourse.bass_utils import run_bass_kernel_spmd  # noqa: E402

BF16 = ml_dtypes.bfloat16

N_CORES = 8
E, C, H, W = 512, 64, 28, 28
HW = H * W                      # 784
HWP = 896                       # corner-table row pad: 896*2B = 1792B = 7*256
M_PAD = 128                     # local corner slots (64 edges * 2 <= 128)
E_LOC = E // N_CORES            # 64 edges per core
GROW = 6272                     # phase-1 gather elem: 8 channels * 784
NHALF = HW // 2                 # 392-wide matmul chunks

_PROGRAM_CACHE = {}


def _wrap_idxs(idx_flat, n_pad):
    """Pack flat gather indices into the dma_gather wrapped layout:
    [128, n_pad//16] int16 with logical index i at [i%16, i//16],
    replicated across the 8 groups of 16 partitions."""
    assert n_pad % 16 == 0
    w = np.zeros((16, n_pad // 16), dtype=np.int16)
    for i, v in enumerate(idx_flat):
        w[i % 16, i // 16] = v
    return np.tile(w, (8, 1))


def _prepare(x, W_agg, corner_edge_pairs, edge_corner, num_corners):
    x = np.asarray(x, dtype=np.float32)
    W_agg = np.asarray(W_agg, dtype=np.float32)
    cep = np.asarray(corner_edge_pairs).astype(np.int64)
    ec = np.asarray(edge_corner).astype(np.int64)
    ncorn = int(num_corners)
    assert x.shape == (E, C, H, W), x.shape

    # reference semantics: scatter drops out-of-range segments, gathers clamp
    seg = cep[:, 0]
    eid = np.clip(cep[:, 1], 0, E - 1)
    valid = (seg >= 0) & (seg < ncorn)
    seg_v, eid_v = seg[valid], eid[valid]
    ec_cl = np.clip(ec, 0, max(ncorn - 1, 0))

    counts = np.bincount(seg_v, minlength=max(ncorn, 1)).astype(np.int64)
    inv_count = 1.0 / np.maximum(counts, 1).astype(np.float64)

    xbf = x.reshape(E, C * HW).astype(BF16)
    xf = xbf.reshape(E * 8, GROW)               # 8-channel gather rows

    # block-diagonal weights for 2-edge batched conv matmuls
    wblk = np.zeros((4, 128, 128), dtype=BF16)
    for t in range(4):
        wt = W_agg[:, t * 64:(t + 1) * 64].T.astype(BF16)    # [c, o]
        wblk[t, :64, :64] = wt
        wblk[t, 64:, 64:] = wt
    wblk_in = wblk.reshape(512, 128)

    # cluster edges onto cores so each core's edge set shares corners:
    # fewer distinct corners => fewer unique incident x rows to gather and
    # fewer scatter-matmul K chunks.  greedy growth by min-new-corners.
    cedges = {}
    for cc, ee in zip(seg_v, eid_v):
        cedges.setdefault(int(cc), set()).add(int(ee))
    unassigned = set(range(E))
    groups = []
    for b in range(N_CORES):
        g = []
        gcorners = set()
        seed = min(unassigned)
        g.append(seed)
        unassigned.discard(seed)
        gcorners |= {int(ec_cl[seed][0]), int(ec_cl[seed][1])}
        while len(g) < E_LOC:
            cands = set()
            for cc in gcorners:
                cands |= (cedges.get(cc, set()) & unassigned)
            if not cands:
                cands = unassigned
            best, bestkey = None, None
            for e in cands:
                c1, c2 = int(ec_cl[e][0]), int(ec_cl[e][1])
                new = (c1 not in gcorners) + (c2 not in gcorners and c1 != c2)
                if bestkey is None or new < bestkey:
                    bestkey, best = new, e
                    if new == 0:
                        break
            g.append(best)
            unassigned.discard(best)
            gcorners |= {int(ec_cl[best][0]), int(ec_cl[best][1])}
        groups.append(np.array(g, dtype=np.int64))

    # per-core: distinct corners, unique incident edges, scatter matrix
    per_core_pre = []
    u_max = 0
    for b in range(N_CORES):
        edges_b = groups[b]
        corners = np.unique(ec_cl[edges_b])
        corners = corners[(corners >= 0) & (corners < max(ncorn, 1))]
        n_idx = {int(c): i for i, c in enumerate(corners)}
        assert len(corners) <= M_PAD
        pmask = np.isin(seg_v, corners)
        p_seg, p_eid = seg_v[pmask], eid_v[pmask]
        uniq = np.unique(p_eid)
        u_idx = {int(e): i for i, e in enumerate(uniq)}
        u_max = max(u_max, len(uniq))
        per_core_pre.append((corners, n_idx, p_seg, p_eid, u_idx, uniq))

    k_chunks = max(1, -(-u_max // 128))
    k_pad = 128 * k_chunks

    per_core = []
    for b in range(N_CORES):
        edges_b = groups[b]
        corners, n_idx, p_seg, p_eid, u_idx, uniq = per_core_pre[b]

        S = np.zeros((k_pad, M_PAD), dtype=np.float32)
        for cc, ee in zip(p_seg, p_eid):
            S[u_idx[int(ee)], n_idx[int(cc)]] += inv_count[cc]

        # stage-1 gather indices: per (kc, j): 128 idxs = eid*8 + j, padded
        # with edge 0 (real data; zero S rows nullify the contribution —
        # never pad with -1: skipped idxs leave stale SBUF and 0*NaN = NaN)
        s1_cols = []
        for kc in range(k_chunks):
            ids = np.zeros(128, dtype=np.int64)
            real = uniq[kc * 128:(kc + 1) * 128]
            ids[:len(real)] = real
            for j in range(8):
                s1_cols.append(_wrap_idxs((ids * 8 + j).astype(np.int16), 128))
        s1i = np.concatenate(s1_cols, axis=1)   # [128, k_chunks*64] int16

        # stage-4 gather indices: 8 instructions x 1024 idxs into the LOCAL
        # corner table: flat[s*128 + m*64 + c] = n_idx(corner)*64 + c
        s4_cols = []
        for binstr in range(8):
            flat = np.zeros(1024, dtype=np.int64)
            for ep in range(4):
                for t in range(2):
                    s = ep * 2 + t
                    for m in range(2):
                        el = m * 32 + binstr * 4 + ep
                        corner = int(ec_cl[edges_b[el], t])
                        base = n_idx.get(corner, 0) * 64
                        i0 = s * 128 + m * 64
                        flat[i0:i0 + 64] = base + np.arange(64)
            s4_cols.append(_wrap_idxs(flat.astype(np.int16), 1024))
        s4i = np.concatenate(s4_cols, axis=1)   # [128, 512] int16

        # local x, SBUF layout: [p = m*64+c, e32*784 + w]
        xl = (xbf[edges_b]
              .reshape(2, 32, C, HW)            # (m, e32, c, w)
              .transpose(0, 2, 1, 3)            # (m, c, e32, w)
              .reshape(128, 32 * HW))
        xl = np.ascontiguousarray(xl)

        per_core.append(dict(
            S=S.astype(BF16),
            s1i=s1i,
            s4i=s4i,
            xl=xl,
            edges=edges_b,
        ))

    return xf, wblk_in, per_core, k_chunks


# --------------------------------------------------------------------------
# device program
# --------------------------------------------------------------------------

def _build_program(k_chunks):
    bf = mybir.dt.bfloat16
    f32 = mybir.dt.float32
    i16 = mybir.dt.int16

    nc = bacc.Bacc("TRN2", target_bir_lowering=False, debug=False,
                   num_devices=N_CORES)

    xf_t = nc.dram_tensor("xf", [E * 8, GROW], bf, kind="ExternalInput").ap()
    xl_t = nc.dram_tensor("xl", [128, 32 * HW], bf, kind="ExternalInput").ap()
    wb_t = nc.dram_tensor("wb", [512, 128], bf, kind="ExternalInput").ap()
    sc_t = nc.dram_tensor("sc", [128 * k_chunks, M_PAD], bf, kind="ExternalInput").ap()
    s1_t = nc.dram_tensor("s1i", [128, k_chunks * 64], i16, kind="ExternalInput").ap()
    s4_t = nc.dram_tensor("s4i", [128, 512], i16, kind="ExternalInput").ap()
    out_t = nc.dram_tensor("out", [128, 32 * HW], bf, kind="ExternalOutput").ap()

    with tile.TileContext(nc) as tc:
        with tc.tile_pool(name="dram", bufs=1, space="DRAM") as dram, \
             tc.tile_pool(name="consts", bufs=1) as consts:
            cft = dram.tile([M_PAD * C, HWP], bf)         # local corner table
            gmx_in = dram.tile([64, HW], bf)
            gmx_out = dram.tile([64, HW], bf, addr_space="Shared")

            # constants (gather indices first so phase-1 gathers start ASAP)
            s1tile = consts.tile([128, k_chunks * 64], i16)
            nc.gpsimd.dma_start(out=s1tile[:], in_=s1_t[:])
            sctiles = []
            for kc in range(k_chunks):
                st = consts.tile([128, M_PAD], bf, tag=f"sc{kc}")
                nc.gpsimd.dma_start(out=st[:], in_=sc_t[kc * 128:(kc + 1) * 128, :])
                sctiles.append(st)
            s4tile = consts.tile([128, 512], i16)
            nc.sync.dma_start(out=s4tile[:], in_=s4_t[:])
            wtiles = []
            for t in range(4):
                wt = consts.tile([128, 128], bf, tag=f"w{t}")
                nc.sync.dma_start(out=wt[:], in_=wb_t[t * 128:(t + 1) * 128, :])
                wtiles.append(wt)
            gm2 = consts.tile([128, HW], bf, tag="gm2")
            gsb = consts.tile([128, HW], f32, tag="gsb")

            # local x, kept in SBUF through phase 4 (loaded AFTER the
            # phase-1 gather-gens below: its 6.4MB would otherwise occupy the
            # DMA engines and delay the tiny s1 index load that gates them)
            xkeep = consts.tile([128, 32 * HW], bf, tag="xkeep")

            # ---------------- phase 1: build local corner table -----------
            with tc.tile_pool(name="p1", bufs=min(3 * k_chunks + 1, 7)) as p1, \
                 tc.tile_pool(name="p1s", bufs=2) as p1s, \
                 tc.tile_pool(name="p2", bufs=1) as p2, \
                 tc.tile_pool(name="psum1", bufs=8, space="PSUM") as psum1:
                # PE warmup: the HAM clock governor sits at K=4 (1.2 GHz)
                # through the DMA-bound start; ~56 dummy matmuls on garbage
                # bits escalate it to K=8 before the real MM stream begins.
                # Numerics are irrelevant; the last tile is sunk to DRAM so
                # the chain is not dead-code-eliminated.
                wsink = dram.tile([128, NHALF], f32)
                pwlast = None
                for wi in range(56):
                    pw = psum1.tile([128, NHALF], f32, space="PSUM", tag="ps1",
                                    name=f"pw{wi}")
                    nc.tensor.matmul(out=pw[:, 0:M_PAD], lhsT=sctiles[0][:],
                                     rhs=sctiles[k_chunks - 1][:],
                                     start=True, stop=True)
                    pwlast = pw
                wsb = p2.tile([128, NHALF], f32, tag="wsb")
                nc.vector.tensor_copy(out=wsb[:], in_=pwlast[:])
                nc.sync.dma_start(out=wsink[:], in_=wsb[:])

                gts = {}
                for j in range(8):
                    for kc in range(k_chunks):
                        gt = p1.tile([128, GROW], bf, tag="gt",
                                     name=f"gt_{kc}_{j}")
                        nc.gpsimd.dma_gather(
                            gt[:].rearrange("p (s d) -> p s d", d=GROW),
                            xf_t[:],
                            s1tile[:, (kc * 8 + j) * 8:(kc * 8 + j) * 8 + 8],
                            num_idxs=128, num_idxs_reg=128,
                            elem_size=GROW,
                        )
                        gts[(kc, j)] = gt

                nc.scalar.dma_start(out=xkeep[:], in_=xl_t[:])

                # phase 2: local max tree on DVE (emitted after the xkeep
                # load it reads), then the AllReduce trigger.  The collective
                # instruction blocks the GpSimd queue on the gmx semaphore,
                # so it must come after all gather descriptor-gens.
                scr = p2.tile([128, 16 * HW], bf, tag="scr")
                nc.vector.tensor_tensor(
                    out=scr[:], in0=xkeep[:, 0:16 * HW],
                    in1=xkeep[:, 16 * HW:32 * HW],
                    op=mybir.AluOpType.max)
                wdt = 8 * HW
                while wdt >= HW:
                    nc.vector.tensor_tensor(
                        out=scr[:, 0:wdt], in0=scr[:, 0:wdt],
                        in1=scr[:, wdt:2 * wdt],
                        op=mybir.AluOpType.max)
                    wdt //= 2
                halfm = p2.tile([64, HW], bf, tag="halfm")
                nc.sync.dma_start(out=halfm[:], in_=scr[64:128, 0:HW])
                nc.vector.tensor_tensor(
                    out=scr[0:64, 0:HW], in0=scr[0:64, 0:HW],
                    in1=halfm[:], op=mybir.AluOpType.max)
                nc.sync.dma_start(out=gmx_in[:], in_=scr[0:64, 0:HW])

                nc.gpsimd.collective_compute(
                    "AllReduce", mybir.AluOpType.max,
                    replica_groups=[list(range(N_CORES))],
                    ins=[gmx_in.opt()], outs=[gmx_out.opt()],
                )

                for j in range(8):
                    # scatter matmuls for channel group j: full 128-corner M.
                    # kc outer within each 4-chunk group => runs of 4
                    # consecutive matmuls share the same stationary weights
                    stg = p1s.tile([128, 8 * HWP], bf, tag="stg")
                    for ng in range(4):
                        pss = [psum1.tile([128, NHALF], f32, space="PSUM",
                                          tag="ps1", name=f"ps1_{j}_{ng}_{i}")
                               for i in range(4)]
                        for kc in range(k_chunks):
                            for n4 in range(4):
                                n = ng * 4 + n4
                                nc.tensor.matmul(
                                    out=pss[n4][:],
                                    lhsT=sctiles[kc][:],
                                    rhs=gts[(kc, j)][:, n * NHALF:(n + 1) * NHALF],
                                    start=(kc == 0), stop=(kc == k_chunks - 1),
                                )
                        for n4 in range(4):
                            n = ng * 4 + n4
                            c8, half = n // 2, n % 2
                            dst = stg[:, c8 * HWP + half * NHALF:
                                      c8 * HWP + half * NHALF + NHALF]
                            # alternate cast engine so neither DVE nor ACT paces
                            if n % 2 == 0:
                                nc.vector.tensor_copy(out=dst, in_=pss[n4][:])
                            else:
                                nc.scalar.activation(
                                    dst, pss[n4][:],
                                    mybir.ActivationFunctionType.Copy)
                    # staging -> local table rows n*64 + j*8 + c8
                    nc.sync.dma_start(
                        out=cft[:].rearrange(
                            "(m j c8) w -> j m (c8 w)", m=M_PAD, j=8, c8=8)[j],
                        in_=stg[:],
                    )

                for wi in range(96):
                    pw2 = psum1.tile([128, NHALF], f32, space="PSUM", tag="ps1",
                                     name=f"pw2_{wi}")
                    nc.tensor.matmul(out=pw2[:], lhsT=wtiles[0][:],
                                     rhs=s4tile[:, 0:NHALF].bitcast(bf),
                                     start=True, stop=True)
                    pwlast = pw2
                wsb2 = p2.tile([128, NHALF], f32, tag="wsb2")
                nc.vector.tensor_copy(out=wsb2[:], in_=pwlast[:])
                nc.sync.dma_start(out=wsink[:], in_=wsb2[:])

            # global max -> gm2 both halves, then G = W4blk @ gmax
            with tc.tile_pool(name="psumg", bufs=2, space="PSUM") as psumg:
                nc.sync.dma_start(out=gm2[0:64, :], in_=gmx_out[:])
                nc.sync.dma_start(out=gm2[64:128, :], in_=gmx_out[:])
                for hh in range(2):
                    sl = slice(hh * NHALF, (hh + 1) * NHALF)
                    psg = psumg.tile([128, NHALF], f32, space="PSUM", tag="psg")
                    nc.tensor.matmul(out=psg[:], lhsT=wtiles[3][:],
                                     rhs=gm2[:, sl], start=True, stop=True)
                    nc.vector.tensor_copy(out=gsb[:, sl], in_=psg[:])

            # ---------------- phase 4: conv -------------------------------
            with tc.tile_pool(name="p4", bufs=6) as p4, \
                 tc.tile_pool(name="p4o", bufs=4) as p4o, \
                 tc.tile_pool(name="p4t", bufs=6) as p4t, \
                 tc.tile_pool(name="psum4", bufs=8, space="PSUM") as psum4:
                for binstr in range(8):
                    lrt = p4.tile([128, 8 * HWP], bf, tag="lrt")
                    nc.gpsimd.dma_gather(
                        lrt[:].rearrange("p (s d) -> p s d", d=HWP),
                        cft[:],
                        s4tile[:, binstr * 64:(binstr + 1) * 64],
                        num_idxs=1024, num_idxs_reg=1024, elem_size=HWP,
                    )
                    # weight-grouped matmul order: runs of 8 share lhsT
                    pss = {}
                    for ep in range(4):
                        for hh in range(2):
                            pss[(ep, hh)] = psum4.tile(
                                [128, NHALF], f32, space="PSUM", tag="ps4",
                                name=f"ps4_{binstr}_{ep}_{hh}")
                    for ep in range(4):
                        e32 = binstr * 4 + ep
                        for hh in range(2):
                            nc.tensor.matmul(out=pss[(ep, hh)][:], lhsT=wtiles[0][:],
                                             rhs=xkeep[:, e32 * HW + hh * NHALF:
                                                       e32 * HW + hh * NHALF + NHALF],
                                             start=True, stop=False)
                    for ep in range(4):
                        for hh in range(2):
                            nc.tensor.matmul(out=pss[(ep, hh)][:], lhsT=wtiles[1][:],
                                             rhs=lrt[:, (ep * 2) * HWP + hh * NHALF:
                                                     (ep * 2) * HWP + hh * NHALF + NHALF],
                                             start=False, stop=False)
                    for ep in range(4):
                        for hh in range(2):
                            nc.tensor.matmul(out=pss[(ep, hh)][:], lhsT=wtiles[2][:],
                                             rhs=lrt[:, (ep * 2 + 1) * HWP + hh * NHALF:
                                                     (ep * 2 + 1) * HWP + hh * NHALF + NHALF],
                                             start=False, stop=True)
                    for ep in range(4):
                        e32 = binstr * 4 + ep
                        ot = p4o.tile([128, HW], bf, tag="ot")
                        for hh in range(2):
                            sl = slice(hh * NHALF, (hh + 1) * NHALF)
                            # add the hoisted gmax term on DVE, relu on ACT
                            tmp = p4t.tile([128, NHALF], bf, tag="tmp")
                            nc.vector.tensor_tensor(out=tmp[:], in0=pss[(ep, hh)][:],
                                                    in1=gsb[:, sl],
                                                    op=mybir.AluOpType.add)
                            nc.scalar.activation(ot[:, sl], tmp[:],
                                                 mybir.ActivationFunctionType.Relu)
                        nc.sync.dma_start(
                            out=out_t[:, e32 * HW:(e32 + 1) * HW],
                            in_=ot[:],
                        )

    nc.compile()
    return nc


# --------------------------------------------------------------------------
# entry point
# --------------------------------------------------------------------------

def _unpack_out(ob):
    """[128, 32*784] bf16 (p = m*64+c, e32*784+w) -> [64, 64, 28, 28] f32."""
    o = np.asarray(ob, dtype=np.float32).reshape(2, C, 32, HW)
    return o.transpose(0, 2, 1, 3).reshape(E_LOC, C, H, W)


def _run(x, W_agg, corner_edge_pairs, edge_corner, num_corners,
         trace=False):
    xf, wblk_in, per_core, k_chunks = _prepare(
        x, W_agg, corner_edge_pairs, edge_corner, num_corners)

    if k_chunks not in _PROGRAM_CACHE:
        _PROGRAM_CACHE[k_chunks] = _build_program(k_chunks)
    nc = _PROGRAM_CACHE[k_chunks]

    in_maps = [{
        "xf": xf, "xl": pc["xl"], "wb": wblk_in,
        "sc": pc["S"], "s1i": pc["s1i"], "s4i": pc["s4i"],
    } for pc in per_core]

    kwargs = dict(trace=trace)
    if trace:
        kwargs["trace_cores"] = list(range(N_CORES))
    res = run_bass_kernel_spmd(nc, in_maps, list(range(N_CORES)), **kwargs)

    out = np.empty((E, C, H, W), dtype=np.float32)
    for b in range(N_CORES):
        out[per_core[b]["edges"]] = _unpack_out(res.results[b]["out"])
    return out, res


def kernel(x, W_agg, corner_edge_pairs, edge_corner, num_corners):
    out, _ = _run(x, W_agg, corner_edge_pairs, edge_corner, num_corners,
                  trace=False)
    return out


# expose for test harness profiling
def _run_profiled(x, W_agg, corner_edge_pairs, edge_corner, num_corners,
                  trace=True):
    return _run(x, W_agg, corner_edge_pairs, edge_corner, num_corners,
                trace=trace)
